# revision 7
# baseline (speedup 1.0000x reference)
"""MoE adapter kernel for Trainium2 (8 NeuronCores, data-parallel over batch).

Full inputs in, full output out. Internally: shard the 8192-row batch into
1024 rows per core (params replicated), run a Bass/Tile kernel per core, and
concatenate the per-core outputs.

Per-core pipeline (rows processed in 2 blocks of 512):
  1. Load x rows, transpose 128x128 blocks on the PE into x^T (D on partitions),
     rounding to float32r for the expert matmuls. The gate's first layer
     accumulates from the pre-rounded fp32 transpose output so routing is
     computed in full fp32 precision.
  2. Gate: g^T = relu(Wg1^T x^T + bg1) -> logits -> top-2 softmax computed with
     max/mask/exp ops into dense per-expert combine weights [rows, 8].
  3. Per expert: h^T = relu(W1^T x^T + b1) (fp32r matmuls, fp32 accumulate),
     out_e = h W2 + b2, combine acc += w_e * out_e on the vector engine.
"""

import numpy as np

import concourse.mybir as mybir
import concourse.tile as tile
from concourse import bacc
from concourse.bass_utils import run_bass_kernel_spmd
from concourse.masks import make_identity

N_CORES = 8
N_FULL = 8192
ROWS = N_FULL // N_CORES   # 1024 rows per core
RB = 2                     # row blocks per core
RBLK = ROWS // RB          # 512 rows per block
P = 128
RCH = RBLK // P            # 4 row chunks per block
ID_DIM = 128
LLM_DIM = 4096
D = ID_DIM + LLM_DIM       # 4224
KC = D // P                # 33 contraction chunks
H = 1024
MC = H // P                # 8 hidden chunks
OUT = 512
E = 8
GH = 2 * E                 # 16

F32 = mybir.dt.float32
F32R = mybir.dt.float32r
AF = mybir.ActivationFunctionType
ALU = mybir.AluOpType
AX = mybir.AxisListType


def _build():
    nc = bacc.Bacc("TRN2", target_bir_lowering=False, debug=False,
                   num_devices=N_CORES)
    id_emb = nc.declare_dram_parameter("id_emb", [ROWS, ID_DIM], F32, isOutput=False)
    llm_emb = nc.declare_dram_parameter("llm_emb", [ROWS, LLM_DIM], F32, isOutput=False)
    Wg1 = nc.declare_dram_parameter("Wg1", [D, GH], F32, isOutput=False)
    bg1 = nc.declare_dram_parameter("bg1", [GH], F32, isOutput=False)
    Wg2 = nc.declare_dram_parameter("Wg2", [GH, E], F32, isOutput=False)
    bg2 = nc.declare_dram_parameter("bg2", [E], F32, isOutput=False)
    W1 = nc.declare_dram_parameter("W1", [E, D, H], F32R, isOutput=False)
    b1 = nc.declare_dram_parameter("b1", [E, H], F32, isOutput=False)
    W2 = nc.declare_dram_parameter("W2", [E, H, OUT], F32R, isOutput=False)
    b2 = nc.declare_dram_parameter("b2", [E, OUT], F32R, isOutput=False)
    out = nc.declare_dram_parameter("out", [ROWS, OUT], F32, isOutput=True)

    with tile.TileContext(nc) as tc:
        with tc.tile_pool(name="const", bufs=1) as const, \
             tc.tile_pool(name="xT", bufs=1) as xT_pool, \
             tc.tile_pool(name="big", bufs=3) as big, \
             tc.tile_pool(name="w2", bufs=2) as w2p, \
             tc.tile_pool(name="hT", bufs=10) as hp, \
             tc.tile_pool(name="acc", bufs=2) as accp, \
             tc.tile_pool(name="stg", bufs=4) as stg, \
             tc.tile_pool(name="g", bufs=2) as gp, \
             tc.tile_pool(name="small", bufs=2) as smallp, \
             tc.tile_pool(name="ps", bufs=2, space="PSUM") as psp:

            ident = const.tile([P, P], F32, tag="ident")
            make_identity(nc, ident)
            ones_f32 = const.tile([1, P], F32, tag="ones_f32")
            nc.vector.memset(ones_f32, 1.0)
            ones_sb = const.tile([1, P], F32R, tag="ones")
            nc.vector.tensor_copy(ones_sb, ones_f32)
            wg1_sb = const.tile([P, KC, GH], F32, tag="wg1")
            nc.sync.dma_start(out=wg1_sb, in_=Wg1.rearrange("(k p) g -> p k g", p=P))
            wg2_sb = const.tile([GH, E], F32, tag="wg2")
            nc.sync.dma_start(out=wg2_sb, in_=Wg2[:])
            bg1_sb = const.tile([GH, 1], F32, tag="bg1")
            nc.sync.dma_start(out=bg1_sb, in_=bg1.rearrange("(g o) -> g o", o=1))
            bg2_sb = const.tile([1, E], F32, tag="bg2")
            nc.sync.dma_start(out=bg2_sb, in_=bg2.rearrange("(o e) -> o e", o=1))
            b1_sb = const.tile([P, E, MC], F32, tag="b1")
            nc.sync.dma_start(out=b1_sb, in_=b1.rearrange("e (m p) -> p e m", p=P))


            for rb in range(RB):
                xT = xT_pool.tile([P, KC, RBLK], F32R, tag="xT")
                g_sb = gp.tile([GH, RBLK], F32, tag="g")
                dw = gp.tile([P, RCH, E], F32, tag="dw")

                # ---- transpose + gate prologue ----
                for c in range(RCH):
                    r0 = rb * RBLK + c * P
                    xl = big.tile([P, KC, P], F32, tag="big")
                    nc.sync.dma_start(out=xl[:, 0, :], in_=id_emb[r0:r0 + P, :])
                    nc.sync.dma_start(
                        out=xl[:, 1:, :],
                        in_=llm_emb[r0:r0 + P, :].rearrange("r (k f) -> r k f", f=P))
                    gps = psp.tile([GH, P], F32, tag="psg")
                    for k in range(KC):
                        tp = psp.tile([P, P], F32, tag="pss")
                        nc.tensor.transpose(tp, xl[:, k, :], ident)
                        st = stg.tile([P, P], F32, tag="stg")
                        nc.vector.tensor_copy(st, tp)
                        nc.vector.tensor_copy(xT[:, k, c * P:(c + 1) * P], tp)
                        nc.tensor.matmul(gps, wg1_sb[:, k, :], st,
                                         start=(k == 0), stop=(k == KC - 1))
                    nc.scalar.activation(g_sb[:, c * P:(c + 1) * P], gps,
                                         AF.Relu, bias=bg1_sb)

                    # logits for this row chunk
                    lt = psp.tile([P, P], F32, tag="pss")
                    nc.tensor.matmul(lt[:, :E], g_sb[:, c * P:(c + 1) * P], wg2_sb,
                                     start=True, stop=False)
                    nc.tensor.matmul(lt[:, :E], ones_f32,
                                     bg2_sb, start=False, stop=True)

                    # top-2 softmax -> dense combine weights dw[:, c, :]
                    lg = lt[:, :E]
                    m1 = smallp.tile([P, 1], F32, tag="m1")
                    nc.vector.tensor_reduce(m1, lg, axis=AX.X, op=ALU.max)
                    eq1 = smallp.tile([P, E], F32, tag="eq1")
                    nc.vector.tensor_scalar(eq1, lg, m1, None, op0=ALU.is_equal)
                    msk = smallp.tile([P, E], F32, tag="msk")
                    nc.vector.scalar_tensor_tensor(msk, eq1, -1e30, lg,
                                                   op0=ALU.mult, op1=ALU.add)
                    m2 = smallp.tile([P, 1], F32, tag="m2")
                    nc.vector.tensor_reduce(m2, msk, axis=AX.X, op=ALU.max)
                    eq2 = smallp.tile([P, E], F32, tag="eq2")
                    nc.vector.tensor_scalar(eq2, msk, m2, None, op0=ALU.is_equal)
                    dd = smallp.tile([P, 1], F32, tag="dd")
                    nc.vector.tensor_sub(dd, m2, m1)
                    ed = smallp.tile([P, 1], F32, tag="ed")
                    nc.scalar.activation(ed, dd, AF.Exp)
                    den = smallp.tile([P, 1], F32, tag="den")
                    nc.vector.tensor_scalar_add(den, ed, 1.0)
                    rr = smallp.tile([P, 1], F32, tag="rr")
                    nc.vector.reciprocal(rr, den)
                    w2v = smallp.tile([P, 1], F32, tag="w2v")
                    nc.vector.tensor_mul(w2v, ed, rr)
                    t1 = smallp.tile([P, E], F32, tag="t1")
                    nc.vector.tensor_scalar(t1, eq1, rr, None, op0=ALU.mult)
                    nc.vector.scalar_tensor_tensor(dw[:, c, :], eq2, w2v, t1,
                                                   op0=ALU.mult, op1=ALU.add)

                # ---- expert loop ----
                accs = [None] * RCH
                for e in range(E):
                    w2t = w2p.tile([P, MC, OUT], F32R, tag="w2")
                    nc.sync.dma_start(
                        out=w2t, in_=W2[e].rearrange("(m p) o -> p m o", p=P))
                    b2row = stg.tile([1, OUT], F32R, tag="b2row")
                    nc.sync.dma_start(
                        out=b2row, in_=b2[e].rearrange("(o f) -> o f", o=1))
                    hts = []
                    w1r = W1[e].rearrange("(k p) h -> p k h", p=P)
                    for m in range(MC):
                        w1t = big.tile([P, KC, P], F32R, tag="big")
                        nc.sync.dma_start(out=w1t,
                                          in_=w1r[:, :, m * P:(m + 1) * P])
                        ph = psp.tile([P, RBLK], F32, tag="psh")
                        for k in range(KC):
                            nc.tensor.matmul(ph, w1t[:, k, :], xT[:, k, :],
                                             start=(k == 0), stop=(k == KC - 1))
                        ht = hp.tile([P, RBLK], F32R, tag="hT")
                        nc.scalar.activation(ht, ph, AF.Relu,
                                             bias=b1_sb[:, e, m:m + 1])
                        hts.append(ht)
                    for c in range(RCH):
                        po = psp.tile([P, OUT], F32, tag="pso")
                        for m in range(MC):
                            nc.tensor.matmul(po, hts[m][:, c * P:(c + 1) * P],
                                             w2t[:, m, :],
                                             start=(m == 0), stop=False)
                        nc.tensor.matmul(po, ones_sb[0:1, :], b2row,
                                         start=False, stop=True)
                        wcol = dw[:, c, e:e + 1]
                        if e == 0:
                            acc = accp.tile([P, OUT], F32, tag=f"acc{c}")
                            accs[c] = acc
                            nc.vector.tensor_scalar(acc, po, wcol, None,
                                                    op0=ALU.mult)
                        else:
                            nc.vector.scalar_tensor_tensor(accs[c], po, wcol,
                                                           accs[c],
                                                           op0=ALU.mult,
                                                           op1=ALU.add)
                for c in range(RCH):
                    r0 = rb * RBLK + c * P
                    nc.sync.dma_start(out=out[r0:r0 + P, :], in_=accs[c])

    nc.compile()
    return nc


_NC_CACHE = None


def kernel(id_emb, llm_emb, Wg1, bg1, Wg2, bg2, W1, b1, W2, b2):
    global _NC_CACHE
    if _NC_CACHE is None:
        _NC_CACHE = _build()
    nc = _NC_CACHE

    id_emb = np.ascontiguousarray(np.asarray(id_emb, dtype=np.float32))
    llm_emb = np.ascontiguousarray(np.asarray(llm_emb, dtype=np.float32))
    shared = {
        "Wg1": np.ascontiguousarray(np.asarray(Wg1, np.float32)),
        "bg1": np.ascontiguousarray(np.asarray(bg1, np.float32)),
        "Wg2": np.ascontiguousarray(np.asarray(Wg2, np.float32)),
        "bg2": np.ascontiguousarray(np.asarray(bg2, np.float32)),
        "W1": np.ascontiguousarray(np.asarray(W1, np.float32)),
        "b1": np.ascontiguousarray(np.asarray(b1, np.float32)),
        "W2": np.ascontiguousarray(np.asarray(W2, np.float32)),
        "b2": np.ascontiguousarray(np.asarray(b2, np.float32)),
    }
    in_maps = []
    for c in range(N_CORES):
        r0 = c * ROWS
        m = dict(shared)
        m["id_emb"] = id_emb[r0:r0 + ROWS]
        m["llm_emb"] = llm_emb[r0:r0 + ROWS]
        in_maps.append(m)

    res = run_bass_kernel_spmd(nc, in_maps, list(range(N_CORES)))
    return np.concatenate([res.results[c]["out"] for c in range(N_CORES)], axis=0)


# revision 8
# speedup vs baseline: 179.9187x; 179.9187x over previous
"""MoE adapter kernel for Trainium2 (8 NeuronCores, data-parallel over batch).

Full inputs in, full output out. Internally: shard the 8192-row batch into
1024 rows per core (params replicated), run a Bass/Tile kernel per core, and
concatenate the per-core outputs.

Per-core pipeline (rows processed in 2 blocks of 512):
  1. Load x rows, transpose 128x128 blocks on the PE into x^T (D on partitions),
     rounding to float32r for the expert matmuls. The gate's first layer
     accumulates from the pre-rounded fp32 transpose output so routing is
     computed in full fp32 precision.
  2. Gate: g^T = relu(Wg1^T x^T + bg1) -> logits -> top-2 softmax computed with
     max/mask/exp ops into dense per-expert combine weights [rows, 8].
  3. Per expert: h^T = relu(W1^T x^T + b1) (fp32r matmuls, fp32 accumulate),
     out_e = h W2 + b2, combine acc += w_e * out_e on the vector engine.
"""

import numpy as np

import concourse.mybir as mybir
import concourse.tile as tile
from concourse import bacc
from concourse.bass_utils import run_bass_kernel_spmd
from concourse.masks import make_identity

N_CORES = 8
N_FULL = 8192
ROWS = N_FULL // N_CORES   # 1024 rows per core
RB = 2                     # row blocks per core
RBLK = ROWS // RB          # 512 rows per block
P = 128
RCH = RBLK // P            # 4 row chunks per block
ID_DIM = 128
LLM_DIM = 4096
D = ID_DIM + LLM_DIM       # 4224
KC = D // P                # 33 contraction chunks
H = 1024
MC = H // P                # 8 hidden chunks
OUT = 512
E = 8
GH = 2 * E                 # 16

F32 = mybir.dt.float32
F32R = mybir.dt.float32r
AF = mybir.ActivationFunctionType
ALU = mybir.AluOpType
AX = mybir.AxisListType


def _build():
    nc = bacc.Bacc("TRN2", target_bir_lowering=False, debug=False,
                   num_devices=N_CORES)
    id_emb = nc.declare_dram_parameter("id_emb", [ROWS, ID_DIM], F32, isOutput=False)
    llm_emb = nc.declare_dram_parameter("llm_emb", [ROWS, LLM_DIM], F32, isOutput=False)
    Wg1 = nc.declare_dram_parameter("Wg1", [D, GH], F32, isOutput=False)
    bg1 = nc.declare_dram_parameter("bg1", [GH], F32, isOutput=False)
    Wg2 = nc.declare_dram_parameter("Wg2", [GH, E], F32, isOutput=False)
    bg2 = nc.declare_dram_parameter("bg2", [E], F32, isOutput=False)
    W1 = nc.declare_dram_parameter("W1", [E, D, H], F32R, isOutput=False)
    b1 = nc.declare_dram_parameter("b1", [E, H], F32, isOutput=False)
    W2 = nc.declare_dram_parameter("W2", [E, H, OUT], F32R, isOutput=False)
    b2 = nc.declare_dram_parameter("b2", [E, OUT], F32R, isOutput=False)
    out = nc.declare_dram_parameter("out", [ROWS, OUT], F32, isOutput=True)

    with tile.TileContext(nc) as tc:
        with tc.tile_pool(name="const", bufs=1) as const, \
             tc.tile_pool(name="xT", bufs=1) as xT_pool, \
             tc.tile_pool(name="big", bufs=3) as big, \
             tc.tile_pool(name="w2", bufs=2) as w2p, \
             tc.tile_pool(name="hT", bufs=10) as hp, \
             tc.tile_pool(name="acc", bufs=2) as accp, \
             tc.tile_pool(name="stg", bufs=4) as stg, \
             tc.tile_pool(name="g", bufs=2) as gp, \
             tc.tile_pool(name="small", bufs=2) as smallp, \
             tc.tile_pool(name="ps", bufs=2, space="PSUM") as psp:

            ident = const.tile([P, P], F32, tag="ident")
            make_identity(nc, ident)
            ones_f32 = const.tile([1, P], F32, tag="ones_f32")
            nc.vector.memset(ones_f32, 1.0)
            ones_sb = const.tile([1, P], F32R, tag="ones")
            nc.vector.tensor_copy(ones_sb, ones_f32)
            wg1_sb = const.tile([P, KC, GH], F32, tag="wg1")
            nc.sync.dma_start(out=wg1_sb, in_=Wg1.rearrange("(k p) g -> p k g", p=P))
            wg2_sb = const.tile([GH, E], F32, tag="wg2")
            nc.sync.dma_start(out=wg2_sb, in_=Wg2[:])
            bg1_sb = const.tile([GH, 1], F32, tag="bg1")
            nc.sync.dma_start(out=bg1_sb, in_=bg1.rearrange("(g o) -> g o", o=1))
            bg2_sb = const.tile([1, E], F32, tag="bg2")
            nc.sync.dma_start(out=bg2_sb, in_=bg2.rearrange("(o e) -> o e", o=1))
            b1_sb = const.tile([P, E, MC], F32, tag="b1")
            nc.sync.dma_start(out=b1_sb, in_=b1.rearrange("e (m p) -> p e m", p=P))


            for rb in range(RB):
                xT = xT_pool.tile([P, KC, RBLK], F32R, tag="xT")
                g_sb = gp.tile([GH, RBLK], F32, tag="g")
                dw = gp.tile([P, RCH, E], F32, tag="dw")

                # ---- transpose + gate prologue ----
                for c in range(RCH):
                    r0 = rb * RBLK + c * P
                    xl = big.tile([P, KC, P], F32, tag="big")
                    nc.sync.dma_start(out=xl[:, 0, :], in_=id_emb[r0:r0 + P, :])
                    nc.sync.dma_start(
                        out=xl[:, 1:, :],
                        in_=llm_emb[r0:r0 + P, :].rearrange("r (k f) -> r k f", f=P))
                    gps = psp.tile([GH, P], F32, tag="psg")
                    for k in range(KC):
                        tp = psp.tile([P, P], F32, tag="pss")
                        nc.tensor.transpose(tp, xl[:, k, :], ident)
                        st = stg.tile([P, P], F32, tag="stg")
                        nc.vector.tensor_copy(st, tp)
                        nc.vector.tensor_copy(xT[:, k, c * P:(c + 1) * P], tp)
                        nc.tensor.matmul(gps, wg1_sb[:, k, :], st,
                                         start=(k == 0), stop=(k == KC - 1))
                    nc.scalar.activation(g_sb[:, c * P:(c + 1) * P], gps,
                                         AF.Relu, bias=bg1_sb)

                    # logits for this row chunk
                    lt = psp.tile([P, P], F32, tag="pss")
                    nc.tensor.matmul(lt[:, :E], g_sb[:, c * P:(c + 1) * P], wg2_sb,
                                     start=True, stop=False)
                    nc.tensor.matmul(lt[:, :E], ones_f32,
                                     bg2_sb, start=False, stop=True)

                    # top-2 softmax -> dense combine weights dw[:, c, :]
                    lg = lt[:, :E]
                    m1 = smallp.tile([P, 1], F32, tag="m1")
                    nc.vector.tensor_reduce(m1, lg, axis=AX.X, op=ALU.max)
                    eq1 = smallp.tile([P, E], F32, tag="eq1")
                    nc.vector.tensor_scalar(eq1, lg, m1, None, op0=ALU.is_equal)
                    msk = smallp.tile([P, E], F32, tag="msk")
                    nc.vector.scalar_tensor_tensor(msk, eq1, -1e30, lg,
                                                   op0=ALU.mult, op1=ALU.add)
                    m2 = smallp.tile([P, 1], F32, tag="m2")
                    nc.vector.tensor_reduce(m2, msk, axis=AX.X, op=ALU.max)
                    eq2 = smallp.tile([P, E], F32, tag="eq2")
                    nc.vector.tensor_scalar(eq2, msk, m2, None, op0=ALU.is_equal)
                    dd = smallp.tile([P, 1], F32, tag="dd")
                    nc.vector.tensor_sub(dd, m2, m1)
                    ed = smallp.tile([P, 1], F32, tag="ed")
                    nc.scalar.activation(ed, dd, AF.Exp)
                    den = smallp.tile([P, 1], F32, tag="den")
                    nc.vector.tensor_scalar_add(den, ed, 1.0)
                    rr = smallp.tile([P, 1], F32, tag="rr")
                    nc.vector.reciprocal(rr, den)
                    w2v = smallp.tile([P, 1], F32, tag="w2v")
                    nc.vector.tensor_mul(w2v, ed, rr)
                    t1 = smallp.tile([P, E], F32, tag="t1")
                    nc.vector.tensor_scalar(t1, eq1, rr, None, op0=ALU.mult)
                    nc.vector.scalar_tensor_tensor(dw[:, c, :], eq2, w2v, t1,
                                                   op0=ALU.mult, op1=ALU.add)

                # ---- expert loop ----
                accs = [None] * RCH
                for e in range(E):
                    w2t = w2p.tile([P, MC, OUT], F32R, tag="w2")
                    nc.sync.dma_start(
                        out=w2t, in_=W2[e].rearrange("(m p) o -> p m o", p=P))
                    b2row = stg.tile([1, OUT], F32R, tag="b2row")
                    nc.sync.dma_start(
                        out=b2row, in_=b2[e].rearrange("(o f) -> o f", o=1))
                    hts = []
                    w1r = W1[e].rearrange("(k p) h -> p k h", p=P)
                    for m in range(MC):
                        w1t = big.tile([P, KC, P], F32R, tag="big")
                        nc.sync.dma_start(out=w1t,
                                          in_=w1r[:, :, m * P:(m + 1) * P])
                        ph = psp.tile([P, RBLK], F32, tag="psh")
                        for k in range(KC):
                            nc.tensor.matmul(ph, w1t[:, k, :], xT[:, k, :],
                                             start=(k == 0), stop=(k == KC - 1))
                        ht = hp.tile([P, RBLK], F32R, tag="hT")
                        nc.scalar.activation(ht, ph, AF.Relu,
                                             bias=b1_sb[:, e, m:m + 1])
                        hts.append(ht)
                    for c in range(RCH):
                        po = psp.tile([P, OUT], F32, tag="pso")
                        for m in range(MC):
                            nc.tensor.matmul(po, hts[m][:, c * P:(c + 1) * P],
                                             w2t[:, m, :],
                                             start=(m == 0), stop=False)
                        nc.tensor.matmul(po, ones_sb[0:1, :], b2row,
                                         start=False, stop=True)
                        wcol = dw[:, c, e:e + 1]
                        if e == 0:
                            acc = accp.tile([P, OUT], F32, tag=f"acc{c}")
                            accs[c] = acc
                            nc.vector.tensor_scalar(acc, po, wcol, None,
                                                    op0=ALU.mult)
                        else:
                            nc.vector.scalar_tensor_tensor(accs[c], po, wcol,
                                                           accs[c],
                                                           op0=ALU.mult,
                                                           op1=ALU.add)
                for c in range(RCH):
                    r0 = rb * RBLK + c * P
                    nc.sync.dma_start(out=out[r0:r0 + P, :], in_=accs[c])

    nc.compile()
    return nc


_NC_CACHE = None


def kernel(id_emb, llm_emb, Wg1, bg1, Wg2, bg2, W1, b1, W2, b2):
    global _NC_CACHE
    if _NC_CACHE is None:
        _NC_CACHE = _build()
    nc = _NC_CACHE

    id_emb = np.ascontiguousarray(np.asarray(id_emb, dtype=np.float32))
    llm_emb = np.ascontiguousarray(np.asarray(llm_emb, dtype=np.float32))
    shared = {
        "Wg1": np.ascontiguousarray(np.asarray(Wg1, np.float32)),
        "bg1": np.ascontiguousarray(np.asarray(bg1, np.float32)),
        "Wg2": np.ascontiguousarray(np.asarray(Wg2, np.float32)),
        "bg2": np.ascontiguousarray(np.asarray(bg2, np.float32)),
        "W1": np.ascontiguousarray(np.asarray(W1, np.float32)),
        "b1": np.ascontiguousarray(np.asarray(b1, np.float32)),
        "W2": np.ascontiguousarray(np.asarray(W2, np.float32)),
        "b2": np.ascontiguousarray(np.asarray(b2, np.float32)),
    }
    in_maps = []
    for c in range(N_CORES):
        r0 = c * ROWS
        m = dict(shared)
        m["id_emb"] = id_emb[r0:r0 + ROWS]
        m["llm_emb"] = llm_emb[r0:r0 + ROWS]
        in_maps.append(m)

    global _last_in_maps
    _last_in_maps = in_maps
    res = run_bass_kernel_spmd(nc, in_maps, list(range(N_CORES)))
    return np.concatenate([res.results[c]["out"] for c in range(N_CORES)], axis=0)


_last_in_maps = None
